# revision 3
# baseline (speedup 1.0000x reference)
"""GCN 2-layer kernel for TRN2, 8 NeuronCores (SPMD, dst-node sharded), v2.

Differences vs v1 baseline:
  - table1 (h1 = x_hat @ W1, dinv-folded) computed FULLY on every core from a
    replicated xT input -> no 3.2MB AllGather, no z-permute phase.
  - layer-2 applies W2 BEFORE the collective: AllGather payload is [12544, 2]
    f32 (100KB) instead of [12544, 64].
  - table2 rows are 64-wide f32 with only cols 0:2 meaningful (gather elem
    must be 256B); cols 2:64 are garbage and never read by the reduces.
  - table row numbering = per-core degree-sorted rank -> no permutes anywhere.
  - schedule: exact (dA, dB) sort, G=1 padding, common cross-core group max,
    chunk A = src cores 0-4 / chunk B = cores 5-7 (signed int16 idx windows).
  - dummy rows (rank 12500..12543 per core) are all-zero -> used as pad rows.
"""
import sys
sys.path.insert(0, "/opt/trn_rl_repo")
import numpy as np
import ml_dtypes

N = 100000
E = 3200000
NCORES = 8
NSH = 12500
SLOTS = 12544
SLOTS_P = SLOTS // 128   # 98 groups
TBL = SLOTS * NCORES     # 100352
NI = 1024                # tokens per gather window
WPOS = 8                 # positions per window
TILE_POS = 128           # positions per region tile (16 windows)
ROWS_A = 5 * SLOTS       # 62720 ; chunk A = rows [0, ROWS_A)
BASE_A = ROWS_A // 2     # 31360
BASE_B = ROWS_A + (TBL - ROWS_A) // 2   # 81536
PAD_ROW_A = 4 * SLOTS + 12540   # core-4 dummy row (always zero)
PAD_ROW_B = 7 * SLOTS + 12540   # core-7 dummy row
PAD_IDX_A = PAD_ROW_A - BASE_A  # positive int16
PAD_IDX_B = PAD_ROW_B - BASE_B  # positive int16
MAXCOLS = 16             # max node-columns per reduce piece


def _csr_expand(counts):
    """within-segment offsets 0..c-1 for each segment, concatenated."""
    total = int(counts.sum())
    if total == 0:
        return np.zeros(0, np.int64)
    ends = np.cumsum(counts)
    idx = np.arange(total, dtype=np.int64)
    seg = np.searchsorted(ends, idx, side="right")
    starts = ends - counts
    return idx - starts[seg]


def _preprocess(edge_index):
    row = np.asarray(edge_index[0], dtype=np.int64)
    col = np.asarray(edge_index[1], dtype=np.int64)
    rows_all = np.concatenate([row, np.arange(N, dtype=np.int64)])
    cols_all = np.concatenate([col, np.arange(N, dtype=np.int64)])
    order = np.argsort(cols_all, kind="stable")
    rows_s = rows_all[order]
    cols_s = cols_all[order]
    starts = np.searchsorted(cols_s, np.arange(N))
    ends = np.searchsorted(cols_s, np.arange(N), side="right")
    deg = (ends - starts).astype(np.float64)
    dinv = (1.0 / np.sqrt(deg)).astype(np.float32)

    isA_tok = rows_s < 5 * NSH          # src core < 5  -> chunk A
    dA = np.bincount(cols_s[isA_tok], minlength=N).astype(np.int64)
    dB = np.bincount(cols_s[~isA_tok], minlength=N).astype(np.int64)

    # per-core sorted rank order (rank -> node id); dummies at rank>=12500
    nodes_all = np.full((NCORES, SLOTS), -1, np.int64)
    rank_of = np.zeros(N, np.int64)
    for p in range(NCORES):
        v0 = p * NSH
        vs = np.arange(v0, v0 + NSH)
        key = dA[vs] * 100000 + dB[vs]
        srt = np.argsort(key, kind="stable")
        nodes_all[p, :NSH] = vs[srt]
        rank_of[vs[srt]] = np.arange(NSH)
    grow = (np.arange(N) // NSH) * SLOTS + rank_of   # node -> table row

    # group maxima (cross-core common schedule)
    def group_max(d):
        D = np.zeros(SLOTS_P, np.int64)
        for p in range(NCORES):
            nf = nodes_all[p]
            dv = np.where(nf >= 0, d[np.clip(nf, 0, N - 1)], 0)
            D = np.maximum(D, dv.reshape(SLOTS_P, 128).max(1))
        return D
    DA = group_max(dA)
    DB = group_max(dB)

    # per-chunk CSR of wrapped idx values, dst-major (cols_s is dst-sorted)
    rs_grow = grow[rows_s]
    valsA = (rs_grow[isA_tok] - BASE_A)       # grouped by dst in order
    valsB = (rs_grow[~isA_tok] - BASE_B)
    startsA = np.concatenate([[0], np.cumsum(dA)])[:-1]
    startsB = np.concatenate([[0], np.cumsum(dB)])[:-1]

    def build_stream(p, gposA, gposB, posA_pad, pos_total):
        st = np.zeros((128, pos_total), np.int64)
        st[:, :posA_pad] = PAD_IDX_A
        st[:, posA_pad:] = PAD_IDX_B
        nf = nodes_all[p]
        real = nf >= 0
        ranks = np.arange(SLOTS)[real]
        vsr = nf[real]
        parts = ranks % 128
        gs = ranks // 128
        for (dv, gpos, vals, st0) in ((dA, gposA, valsA, startsA),
                                      (dB, gposB, valsB, startsB)):
            cnt = dv[vsr]
            off = _csr_expand(cnt)
            rowi = np.repeat(parts, cnt)
            posi = np.repeat(gpos[gs], cnt) + off
            srci = np.repeat(st0[vsr], cnt) + off
            st[rowi, posi] = vals[srci]
        return st

    # ---- schedule + streams; iterate on tail-positivity bumps ----
    for _bump_iter in range(200):
        gposA = np.full(SLOTS_P, -1, np.int64)
        posA = 0
        for g in range(SLOTS_P):
            if DA[g] > 0:
                gposA[g] = posA
                posA += int(DA[g])
        posA_pad = -(-posA // WPOS) * WPOS
        gposB = np.full(SLOTS_P, -1, np.int64)
        posB = posA_pad
        for g in range(SLOTS_P):
            if DB[g] > 0:
                gposB[g] = posB
                posB += int(DB[g])
        pos_total = -(-posB // WPOS) * WPOS
        W_total = pos_total // WPOS
        nA_win = posA_pad // WPOS

        streams = [build_stream(p, gposA, gposB, posA_pad, pos_total)
                   for p in range(NCORES)]

        # tail positivity: stream[127, 8w+7] >= 0 for every window.
        # The schedule (positions/D) is common, but token order within a
        # node's span is per-core free -> fix each core independently.
        bump_group = None
        for p in range(NCORES):
            s127 = streams[p][127]
            tails = s127[WPOS - 1::WPOS]
            for w in np.where(tails < 0)[0]:
                t = int(WPOS * w + WPOS - 1)
                if t < posA_pad:
                    gp, Dv, chunk = gposA, DA, 0
                else:
                    gp, Dv, chunk = gposB, DB, 1
                gi = np.where((gp >= 0) & (gp <= t) & (t < gp + Dv))[0]
                assert len(gi) == 1, (t, gi)
                g = int(gi[0])
                lo, hi = int(gp[g]), int(gp[g] + Dv[g])
                span = s127[lo:hi]
                cols = np.where(span >= 0)[0]
                cols = [c for c in cols if (lo + c) % WPOS != WPOS - 1]
                if cols:
                    t2 = lo + int(cols[-1])
                    s127[t], s127[t2] = s127[t2], s127[t]
                else:
                    bump_group = (chunk, g)
                    break
            if bump_group is not None:
                break
        if bump_group is None:
            break
        chunk, g = bump_group
        if chunk == 0:
            DA[g] += 1
        else:
            DB[g] += 1
    else:
        raise RuntimeError("tail positivity did not converge")

    # ---- reduce op list (common) ----
    # ops: (tile, off, ncols, d, g0, full) ; full -> whole node columns,
    # otherwise partial accumulation into column g0 only.
    def build_ops(gpos, Dv):
        ops = []
        runs = []  # (g0, ngroups, D)
        g = 0
        while g < SLOTS_P:
            if Dv[g] == 0:
                g += 1
                continue
            d = Dv[g]
            n = 1
            while (g + n < SLOTS_P and Dv[g + n] == d
                   and gpos[g + n] == gpos[g] + n * d):
                n += 1
            runs.append((g, n, int(d)))
            g += n
        for (g0, n, d) in runs:
            P0 = int(gpos[g0])
            # split into tile-bounded, MAXCOLS-bounded full-column pieces +
            # partial pieces at tile boundaries
            col = 0          # node column index within run (0..n*?)
            doff = 0         # token offset within current column
            pos = P0
            end = P0 + n * d
            while pos < end:
                tile = pos // TILE_POS
                room = (tile + 1) * TILE_POS - pos
                if doff > 0:
                    # finish partial column
                    take = min(room, d - doff)
                    ops.append((tile, pos % TILE_POS, 1, take, g0 + col,
                                False))
                    doff += take
                    pos += take
                    if doff == d:
                        doff = 0
                        col += 1
                    continue
                ncols_fit = room // d
                ncols = min(ncols_fit, n - col, MAXCOLS)
                if ncols >= 1:
                    ops.append((tile, pos % TILE_POS, ncols, d, g0 + col,
                                True))
                    col += ncols
                    pos += ncols * d
                else:
                    take = min(room, d)
                    ops.append((tile, pos % TILE_POS, 1, take, g0 + col,
                                False))
                    doff = take
                    pos += take
                    if doff == d:
                        doff = 0
                        col += 1
        return ops
    opsA = build_ops(gposA, DA)
    opsB = build_ops(gposB, DB)

    # wrap idx streams to the dma_gather layout: [16, W*64]
    def wrap(stream):
        out = np.zeros((16, W_total * 64), np.int16)
        for w in range(W_total):
            blk = stream[:, w * WPOS:(w + 1) * WPOS]   # [128 part, 8 pos]
            tokens = blk.T.reshape(-1)                  # j = pos*128+part
            wr = tokens.reshape(64, 16).T
            out[:, w * 64:(w + 1) * 64] = wr.astype(np.int16)
        return out

    idx_streams = [wrap(s) for s in streams]

    n_tiles = pos_total // TILE_POS + (1 if pos_total % TILE_POS else 0)
    return dict(dinv=dinv, nodes_all=nodes_all, grow=grow,
                W_total=W_total, nA_win=nA_win, pos_total=pos_total,
                n_tiles=n_tiles, opsA=opsA, opsB=opsB,
                idx_streams=idx_streams)


def _build(pre, skip_spmm1=False, skip_spmm2=False, skip_coll=False,
           skip_p1=False):
    import concourse.bacc as bacc
    import concourse.mybir as mybir
    import concourse.tile as tile

    W_total = pre["W_total"]
    nA_win = pre["nA_win"]
    pos_total = pre["pos_total"]
    n_tiles = pre["n_tiles"]
    ops_by_tile = {}
    for ops in (pre["opsA"], pre["opsB"]):
        for op in ops:
            ops_by_tile.setdefault(op[0], []).append(op)

    nc = bacc.Bacc("TRN2", target_bir_lowering=False, debug=False,
                   num_devices=NCORES, num_swdge_queues=4)
    dt = mybir.dt
    xT_d = nc.dram_tensor("xT", (128, TBL), dt.bfloat16, kind="ExternalInput")
    W1_d = nc.dram_tensor("W1b", (128, 64), dt.bfloat16, kind="ExternalInput")
    w2_d = nc.dram_tensor("w2rep", (128, 2, 64), dt.float32,
                          kind="ExternalInput")
    dsq_d = nc.dram_tensor("dinvsq", (128, SLOTS_P), dt.float32,
                           kind="ExternalInput")
    din_d = nc.dram_tensor("dinv1", (128, SLOTS_P), dt.float32,
                           kind="ExternalInput")
    idx_d = nc.dram_tensor("idxs", (16, W_total * 64), dt.int16,
                           kind="ExternalInput")
    out_d = nc.dram_tensor("out2", (SLOTS, 2), dt.float32,
                           kind="ExternalOutput")

    IDXCH = 64   # windows per idx chunk load

    def replicate16(it):
        for k in (16, 32, 64):
            nc.sync.dma_start(it[k:2 * k, :], it[0:k, :])

    with tile.TileContext(nc) as tc:
        with tc.tile_pool(name="dram", bufs=1, space="DRAM") as dram, \
             tc.tile_pool(name="const", bufs=1) as constp, \
             tc.tile_pool(name="xtp", bufs=3) as xtp, \
             tc.tile_pool(name="psum", bufs=2, space="PSUM") as psump, \
             tc.tile_pool(name="cp", bufs=3) as cpp, \
             tc.tile_pool(name="regions", bufs=3) as regp, \
             tc.tile_pool(name="idxp", bufs=2) as idxp, \
             tc.tile_pool(name="tmpp", bufs=3) as tmpp, \
             tc.tile_pool(name="nodes", bufs=1) as nodep:

            table1 = dram.tile([TBL, 64], dt.float32)
            table2 = dram.tile([TBL, 64], dt.float32)
            z2loc = dram.tile([SLOTS, 2], dt.float32)
            table2z = dram.tile([TBL, 2], dt.float32, addr_space="Shared")

            W1t = constp.tile([128, 64], dt.bfloat16)
            nc.sync.dma_start(W1t[:], W1_d.ap())
            w2t = constp.tile([128, 2, 64], dt.float32)
            nc.sync.dma_start(w2t[:], w2_d.ap())
            dsqt = constp.tile([128, SLOTS_P], dt.float32)
            nc.sync.dma_start(dsqt[:], dsq_d.ap())
            dint = constp.tile([128, SLOTS_P], dt.float32)
            nc.sync.dma_start(dint[:], din_d.ap())

            # ---- P1: table1 = x_hat @ W1 for ALL rows (local, replicated) --
            t1v = table1[:].rearrange("(n p) f -> n p f", p=128)  # [784,128,64]
            NT = TBL // 128
            for b in ([] if skip_p1 else range(0, NT, 16)):
                nb = min(16, NT - b)
                ps = psump.tile([128, 16, 64], dt.float32, tag="ps")
                xt = xtp.tile([128, nb * 128], dt.bfloat16, tag="xt")
                nc.sync.dma_start(xt[:], xT_d.ap()[:, b * 128:(b + nb) * 128])
                for t in range(nb):
                    nc.tensor.matmul(ps[:, t, :],
                                     lhsT=xt[:, t * 128:(t + 1) * 128],
                                     rhs=W1t[:], start=True, stop=True)
                sb = cpp.tile([128, nb, 64], dt.float32, tag="sb")
                nc.vector.tensor_copy(sb[:], ps[:, :nb, :])
                nc.sync.dma_start(
                    t1v[b:b + nb].rearrange("n p f -> p n f"), sb[:])

            # ---- gather + segmented reduce over one layer ----
            def spmm(table, Sig, nfeat):
                srcA = table[BASE_A:, :]
                srcB = table[BASE_B:, :]
                qn = 0
                reg = None
                for c0 in range(0, W_total, IDXCH):
                    nw = min(IDXCH, W_total - c0)
                    it = idxp.tile([128, nw * 64], dt.int16, tag="idx")
                    nc.sync.dma_start(
                        it[0:16, :], idx_d.ap()[:, c0 * 64:(c0 + nw) * 64])
                    replicate16(it)
                    for j in range(nw):
                        w = c0 + j
                        gpos = w * WPOS
                        tile_i = gpos // TILE_POS
                        off = gpos % TILE_POS
                        if off == 0:
                            reg = regp.tile([128, TILE_POS, 64], dt.float32,
                                            tag="reg")
                        src = srcA if w < nA_win else srcB
                        nc.gpsimd.dma_gather(
                            reg[:, off:off + WPOS, :], src,
                            it[:, j * 64:(j + 1) * 64], NI, NI, 64,
                            queue_num=qn % 4)
                        qn += 1
                        if off + WPOS == TILE_POS or w == W_total - 1:
                            for op in ops_by_tile.get(tile_i, []):
                                (_, o, ncols, d, g0, full) = op
                                rv = reg[:, o:o + ncols * d, 0:nfeat]
                                rv = rv.rearrange("p (n d) f -> p n f d", d=d)
                                tmp = tmpp.tile([128, MAXCOLS, nfeat],
                                                dt.float32, tag=f"tmp{nfeat}")
                                nc.vector.tensor_reduce(
                                    tmp[:, :ncols, :], rv,
                                    mybir.AxisListType.X, mybir.AluOpType.add)
                                o_ = Sig[:, g0:g0 + ncols, :]
                                nc.any.tensor_add(o_, o_, tmp[:, :ncols, :])

            # ---- P2: layer-1 spmm ----
            Sig1 = nodep.tile([128, SLOTS_P, 64], dt.float32, tag="sig1")
            nc.vector.memset(Sig1[:], 0.0)
            if not skip_spmm1:
                spmm(table1, Sig1[:], 64)

            # ---- P3: z2 = (dinv^2 * relu(Sig1)) @ W2 ----
            dv2 = dsqt[:].to_broadcast([128, SLOTS_P, 64])
            nc.vector.tensor_scalar_max(Sig1[:], Sig1[:], 0.0)
            nc.vector.tensor_tensor(Sig1[:], Sig1[:], dv2,
                                    mybir.AluOpType.mult)
            z2t = nodep.tile([128, SLOTS_P, 2], dt.float32, tag="z2")
            for j in range(2):
                w2j = w2t[:, j, :].to_broadcast(
                    [128, 64, SLOTS_P]).rearrange("p f n -> p n f")
                tmpm = regp.tile([128, TILE_POS, 64], dt.float32, tag="reg")
                nc.vector.tensor_tensor(tmpm[:, :SLOTS_P, :], Sig1[:], w2j,
                                        mybir.AluOpType.mult)
                nc.vector.tensor_reduce(z2t[:, :, j], tmpm[:, :SLOTS_P, :],
                                        mybir.AxisListType.X,
                                        mybir.AluOpType.add)
            nc.sync.dma_start(
                z2loc[:].rearrange("(s p) c -> p s c", p=128), z2t[:])

            # ---- P4: AllGather z2 (100KB) ----
            if not skip_coll:
                nc.gpsimd.collective_compute(
                    "AllGather", mybir.AluOpType.bypass,
                    replica_groups=[list(range(NCORES))],
                    ins=[z2loc.opt()], outs=[table2z.opt()])

            # ---- P5: scatter z2 into 64-wide table2 (cols 2:64 garbage) ----
            t2zv = table2z[:].rearrange("(b s p) c -> b p s c", p=128,
                                        s=SLOTS_P)
            t2v = table2[:].rearrange("(b s p) f -> b p s f", p=128,
                                      s=SLOTS_P)
            for b in range(NCORES):
                tz = cpp.tile([128, SLOTS_P, 2], dt.float32, tag="tz")
                nc.sync.dma_start(tz[:], t2zv[b])
                stage = regp.tile([128, TILE_POS, 64], dt.float32, tag="reg")
                nc.vector.tensor_copy(stage[:, :SLOTS_P, 0:2], tz[:])
                nc.sync.dma_start(t2v[b], stage[:, :SLOTS_P, :])

            # ---- P6: layer-2 spmm (only cols 0:2 read) ----
            Sig2 = nodep.tile([128, SLOTS_P, 2], dt.float32, tag="sig2")
            nc.vector.memset(Sig2[:], 0.0)
            if not skip_spmm2:
                spmm(table2, Sig2[:], 2)

            # ---- P7: out = dinv * Sig2 ----
            dv1 = dint[:].to_broadcast([128, SLOTS_P, 2])
            nc.vector.tensor_tensor(Sig2[:], Sig2[:], dv1,
                                    mybir.AluOpType.mult)
            nc.sync.dma_start(
                out_d.ap().rearrange("(s p) c -> p s c", p=128), Sig2[:])

    nc.compile()
    return nc


def kernel(x, edge_index, W1, b1, W2, b2):
    x = np.asarray(x)
    edge_index = np.asarray(edge_index)
    W1 = np.asarray(W1, dtype=np.float32)
    b1 = np.asarray(b1, dtype=np.float32)
    W2 = np.asarray(W2, dtype=np.float32)
    b2 = np.asarray(b2, dtype=np.float32)
    assert np.abs(b1).max() == 0 and np.abs(b2).max() == 0

    pre = _preprocess(edge_index)
    dinv = pre["dinv"]
    nodes_all = pre["nodes_all"]

    # xT: [128, TBL] bf16, column grow(v) = dinv_v * x_v ; dummies zero
    xT = np.zeros((128, TBL), ml_dtypes.bfloat16)
    xs = (x.astype(np.float32) * dinv[:, None]).astype(ml_dtypes.bfloat16)
    xT[:, pre["grow"]] = xs.T

    W1b = W1.astype(ml_dtypes.bfloat16)
    w2rep = np.broadcast_to(W2.T[None, :, :], (128, 2, 64)).copy().astype(
        np.float32)

    nc = _build(pre)

    in_maps = []
    for p in range(NCORES):
        nf = nodes_all[p]
        dsq = np.zeros((128, SLOTS_P), np.float32)
        din = np.zeros((128, SLOTS_P), np.float32)
        valid = nf >= 0
        ranks = np.arange(SLOTS)
        parts, slots = ranks % 128, ranks // 128
        dv = np.where(valid, dinv[np.clip(nf, 0, N - 1)], 0.0)
        dsq[parts, slots] = (dv * dv).astype(np.float32)
        din[parts, slots] = dv.astype(np.float32)
        in_maps.append({
            "xT": np.asarray(xT),
            "W1b": np.asarray(W1b),
            "w2rep": w2rep,
            "dinvsq": dsq,
            "dinv1": din,
            "idxs": pre["idx_streams"][p],
        })

    res = _run(nc, in_maps)

    out = np.zeros((N, 2), np.float32)
    for p in range(NCORES):
        o = res[p]["out2"]          # row r = rank r
        nf = nodes_all[p]
        valid = nf >= 0
        out[nf[valid]] = o[np.arange(SLOTS)[valid]]
    return out.astype(x.dtype if np.issubdtype(x.dtype, np.floating)
                      else np.float32)


def _run(nc, in_maps):
    from concourse import bass_utils
    res = bass_utils.run_bass_kernel_spmd(nc, in_maps,
                                          core_ids=list(range(NCORES)))
    return res.results


# revision 4
# speedup vs baseline: 1.1056x; 1.1056x over previous
"""GCN 2-layer kernel for TRN2, 8 NeuronCores (SPMD, dst-node sharded), v2.

Differences vs v1 baseline:
  - table1 (h1 = x_hat @ W1, dinv-folded) computed FULLY on every core from a
    replicated xT input -> no 3.2MB AllGather, no z-permute phase.
  - layer-2 applies W2 BEFORE the collective: AllGather payload is [12544, 2]
    f32 (100KB) instead of [12544, 64].
  - table2 rows are 64-wide f32 with only cols 0:2 meaningful (gather elem
    must be 256B); cols 2:64 are garbage and never read by the reduces.
  - table row numbering = per-core degree-sorted rank -> no permutes anywhere.
  - schedule: exact (dA, dB) sort, G=1 padding, common cross-core group max,
    chunk A = src cores 0-4 / chunk B = cores 5-7 (signed int16 idx windows).
  - dummy rows (rank 12500..12543 per core) are all-zero -> used as pad rows.
"""
import sys
sys.path.insert(0, "/opt/trn_rl_repo")
import numpy as np
import ml_dtypes

N = 100000
E = 3200000
NCORES = 8
NSH = 12500
SLOTS = 12544
SLOTS_P = SLOTS // 128   # 98 groups
TBL = SLOTS * NCORES     # 100352
NI = 1024                # tokens per gather window
WPOS = 8                 # positions per window
TILE_POS = 128           # positions per region tile (16 windows)
ROWS_A = 5 * SLOTS       # 62720 ; chunk A = rows [0, ROWS_A)
BASE_A = ROWS_A // 2     # 31360
BASE_B = ROWS_A + (TBL - ROWS_A) // 2   # 81536
PAD_ROW_A = 4 * SLOTS + 12540   # core-4 dummy row (always zero)
PAD_ROW_B = 7 * SLOTS + 12540   # core-7 dummy row
PAD_IDX_A = PAD_ROW_A - BASE_A  # positive int16
PAD_IDX_B = PAD_ROW_B - BASE_B  # positive int16
MAXCOLS = 16             # max node-columns per reduce piece


def _csr_expand(counts):
    """within-segment offsets 0..c-1 for each segment, concatenated."""
    total = int(counts.sum())
    if total == 0:
        return np.zeros(0, np.int64)
    ends = np.cumsum(counts)
    idx = np.arange(total, dtype=np.int64)
    seg = np.searchsorted(ends, idx, side="right")
    starts = ends - counts
    return idx - starts[seg]


def _preprocess(edge_index):
    row = np.asarray(edge_index[0], dtype=np.int64)
    col = np.asarray(edge_index[1], dtype=np.int64)
    rows_all = np.concatenate([row, np.arange(N, dtype=np.int64)])
    cols_all = np.concatenate([col, np.arange(N, dtype=np.int64)])
    order = np.argsort(cols_all, kind="stable")
    rows_s = rows_all[order]
    cols_s = cols_all[order]
    starts = np.searchsorted(cols_s, np.arange(N))
    ends = np.searchsorted(cols_s, np.arange(N), side="right")
    deg = (ends - starts).astype(np.float64)
    dinv = (1.0 / np.sqrt(deg)).astype(np.float32)

    isA_tok = rows_s < 5 * NSH          # src core < 5  -> chunk A
    dA = np.bincount(cols_s[isA_tok], minlength=N).astype(np.int64)
    dB = np.bincount(cols_s[~isA_tok], minlength=N).astype(np.int64)

    # per-core sorted rank order (rank -> node id); dummies at rank>=12500
    nodes_all = np.full((NCORES, SLOTS), -1, np.int64)
    rank_of = np.zeros(N, np.int64)
    for p in range(NCORES):
        v0 = p * NSH
        vs = np.arange(v0, v0 + NSH)
        key = dA[vs] * 100000 + dB[vs]
        srt = np.argsort(key, kind="stable")
        nodes_all[p, :NSH] = vs[srt]
        rank_of[vs[srt]] = np.arange(NSH)
    grow = (np.arange(N) // NSH) * SLOTS + rank_of   # node -> table row

    # group maxima (cross-core common schedule)
    def group_max(d):
        D = np.zeros(SLOTS_P, np.int64)
        for p in range(NCORES):
            nf = nodes_all[p]
            dv = np.where(nf >= 0, d[np.clip(nf, 0, N - 1)], 0)
            D = np.maximum(D, dv.reshape(SLOTS_P, 128).max(1))
        return D
    DA = group_max(dA)
    DB = group_max(dB)

    # per-chunk CSR of wrapped idx values, dst-major (cols_s is dst-sorted)
    rs_grow = grow[rows_s]
    valsA = (rs_grow[isA_tok] - BASE_A)       # grouped by dst in order
    valsB = (rs_grow[~isA_tok] - BASE_B)
    startsA = np.concatenate([[0], np.cumsum(dA)])[:-1]
    startsB = np.concatenate([[0], np.cumsum(dB)])[:-1]

    def build_stream(p, gposA, gposB, posA_pad, pos_total):
        st = np.zeros((128, pos_total), np.int64)
        st[:, :posA_pad] = PAD_IDX_A
        st[:, posA_pad:] = PAD_IDX_B
        nf = nodes_all[p]
        real = nf >= 0
        ranks = np.arange(SLOTS)[real]
        vsr = nf[real]
        parts = ranks % 128
        gs = ranks // 128
        for (dv, gpos, vals, st0) in ((dA, gposA, valsA, startsA),
                                      (dB, gposB, valsB, startsB)):
            cnt = dv[vsr]
            off = _csr_expand(cnt)
            rowi = np.repeat(parts, cnt)
            posi = np.repeat(gpos[gs], cnt) + off
            srci = np.repeat(st0[vsr], cnt) + off
            st[rowi, posi] = vals[srci]
        return st

    # ---- schedule + streams; iterate on tail-positivity bumps ----
    for _bump_iter in range(200):
        gposA = np.full(SLOTS_P, -1, np.int64)
        posA = 0
        for g in range(SLOTS_P):
            if DA[g] > 0:
                gposA[g] = posA
                posA += int(DA[g])
        posA_pad = -(-posA // WPOS) * WPOS
        gposB = np.full(SLOTS_P, -1, np.int64)
        posB = posA_pad
        for g in range(SLOTS_P):
            if DB[g] > 0:
                gposB[g] = posB
                posB += int(DB[g])
        pos_total = -(-posB // WPOS) * WPOS
        W_total = pos_total // WPOS
        nA_win = posA_pad // WPOS

        streams = [build_stream(p, gposA, gposB, posA_pad, pos_total)
                   for p in range(NCORES)]

        # tail positivity: stream[127, 8w+7] >= 0 for every window.
        # The schedule (positions/D) is common, but token order within a
        # node's span is per-core free -> fix each core independently.
        bump_group = None
        for p in range(NCORES):
            s127 = streams[p][127]
            tails = s127[WPOS - 1::WPOS]
            for w in np.where(tails < 0)[0]:
                t = int(WPOS * w + WPOS - 1)
                if t < posA_pad:
                    gp, Dv, chunk = gposA, DA, 0
                else:
                    gp, Dv, chunk = gposB, DB, 1
                gi = np.where((gp >= 0) & (gp <= t) & (t < gp + Dv))[0]
                assert len(gi) == 1, (t, gi)
                g = int(gi[0])
                lo, hi = int(gp[g]), int(gp[g] + Dv[g])
                span = s127[lo:hi]
                cols = np.where(span >= 0)[0]
                cols = [c for c in cols if (lo + c) % WPOS != WPOS - 1]
                if cols:
                    t2 = lo + int(cols[-1])
                    s127[t], s127[t2] = s127[t2], s127[t]
                else:
                    bump_group = (chunk, g)
                    break
            if bump_group is not None:
                break
        if bump_group is None:
            break
        chunk, g = bump_group
        if chunk == 0:
            DA[g] += 1
        else:
            DB[g] += 1
    else:
        raise RuntimeError("tail positivity did not converge")

    # ---- reduce op list (common) ----
    # ops: (tile, off, ncols, d, g0, full) ; full -> whole node columns,
    # otherwise partial accumulation into column g0 only.
    def build_ops(gpos, Dv):
        ops = []
        runs = []  # (g0, ngroups, D)
        g = 0
        while g < SLOTS_P:
            if Dv[g] == 0:
                g += 1
                continue
            d = Dv[g]
            n = 1
            while (g + n < SLOTS_P and Dv[g + n] == d
                   and gpos[g + n] == gpos[g] + n * d):
                n += 1
            runs.append((g, n, int(d)))
            g += n
        for (g0, n, d) in runs:
            P0 = int(gpos[g0])
            # split into tile-bounded, MAXCOLS-bounded full-column pieces +
            # partial pieces at tile boundaries
            col = 0          # node column index within run (0..n*?)
            doff = 0         # token offset within current column
            pos = P0
            end = P0 + n * d
            while pos < end:
                tile = pos // TILE_POS
                room = (tile + 1) * TILE_POS - pos
                if doff > 0:
                    # finish partial column
                    take = min(room, d - doff)
                    ops.append((tile, pos % TILE_POS, 1, take, g0 + col,
                                False))
                    doff += take
                    pos += take
                    if doff == d:
                        doff = 0
                        col += 1
                    continue
                ncols_fit = room // d
                ncols = min(ncols_fit, n - col, MAXCOLS)
                if ncols >= 1:
                    ops.append((tile, pos % TILE_POS, ncols, d, g0 + col,
                                True))
                    col += ncols
                    pos += ncols * d
                else:
                    take = min(room, d)
                    ops.append((tile, pos % TILE_POS, 1, take, g0 + col,
                                False))
                    doff = take
                    pos += take
                    if doff == d:
                        doff = 0
                        col += 1
        return ops
    opsA = build_ops(gposA, DA)
    opsB = build_ops(gposB, DB)

    # wrap idx streams to the dma_gather layout: [16, W*64]
    def wrap(stream):
        out = np.zeros((16, W_total * 64), np.int16)
        for w in range(W_total):
            blk = stream[:, w * WPOS:(w + 1) * WPOS]   # [128 part, 8 pos]
            tokens = blk.T.reshape(-1)                  # j = pos*128+part
            wr = tokens.reshape(64, 16).T
            out[:, w * 64:(w + 1) * 64] = wr.astype(np.int16)
        return out

    idx_streams = [wrap(s) for s in streams]

    n_tiles = pos_total // TILE_POS + (1 if pos_total % TILE_POS else 0)
    return dict(dinv=dinv, nodes_all=nodes_all, grow=grow,
                W_total=W_total, nA_win=nA_win, pos_total=pos_total,
                n_tiles=n_tiles, opsA=opsA, opsB=opsB,
                idx_streams=idx_streams)


def _build(pre, skip_spmm1=False, skip_spmm2=False, skip_coll=False,
           skip_p1=False):
    import concourse.bacc as bacc
    import concourse.mybir as mybir
    import concourse.tile as tile

    W_total = pre["W_total"]
    nA_win = pre["nA_win"]
    pos_total = pre["pos_total"]
    n_tiles = pre["n_tiles"]
    ops_by_tile = {}
    for ops in (pre["opsA"], pre["opsB"]):
        for op in ops:
            ops_by_tile.setdefault(op[0], []).append(op)

    nc = bacc.Bacc("TRN2", target_bir_lowering=False, debug=False,
                   num_devices=NCORES, num_swdge_queues=4)
    dt = mybir.dt
    xT_d = nc.dram_tensor("xT", (128, TBL), dt.bfloat16, kind="ExternalInput")
    W1_d = nc.dram_tensor("W1b", (128, 64), dt.bfloat16, kind="ExternalInput")
    w2_d = nc.dram_tensor("w2rep", (128, 2, 64), dt.float32,
                          kind="ExternalInput")
    dsq_d = nc.dram_tensor("dinvsq", (128, SLOTS_P), dt.float32,
                           kind="ExternalInput")
    din_d = nc.dram_tensor("dinv1", (128, SLOTS_P), dt.float32,
                           kind="ExternalInput")
    idx_d = nc.dram_tensor("idxs", (16, W_total * 64), dt.int16,
                           kind="ExternalInput")
    out_d = nc.dram_tensor("out2", (SLOTS, 2), dt.float32,
                           kind="ExternalOutput")

    IDXCH = 64   # windows per idx chunk load

    def replicate16(it):
        for k in (16, 32, 64):
            nc.sync.dma_start(it[k:2 * k, :], it[0:k, :])

    with tile.TileContext(nc) as tc:
        with tc.tile_pool(name="dram", bufs=1, space="DRAM") as dram, \
             tc.tile_pool(name="const", bufs=1) as constp, \
             tc.tile_pool(name="xtp", bufs=3) as xtp, \
             tc.tile_pool(name="psum", bufs=2, space="PSUM") as psump, \
             tc.tile_pool(name="cp", bufs=3) as cpp, \
             tc.tile_pool(name="regions", bufs=3) as regp, \
             tc.tile_pool(name="idxp", bufs=2) as idxp, \
             tc.tile_pool(name="tmpp", bufs=3) as tmpp, \
             tc.tile_pool(name="nodes", bufs=1) as nodep:

            table1 = dram.tile([TBL, 128], dt.bfloat16)
            table2 = dram.tile([TBL, 128], dt.bfloat16)
            z2loc = dram.tile([SLOTS, 2], dt.float32)
            table2z = dram.tile([TBL, 2], dt.float32, addr_space="Shared")

            W1t = constp.tile([128, 64], dt.bfloat16)
            nc.sync.dma_start(W1t[:], W1_d.ap())
            w2t = constp.tile([128, 2, 64], dt.float32)
            nc.sync.dma_start(w2t[:], w2_d.ap())
            dsqt = constp.tile([128, SLOTS_P], dt.float32)
            nc.sync.dma_start(dsqt[:], dsq_d.ap())
            dint = constp.tile([128, SLOTS_P], dt.float32)
            nc.sync.dma_start(dint[:], din_d.ap())

            # ---- P1: table1 = x_hat @ W1 for ALL rows (local, replicated) --
            t1v = table1[:].rearrange("(n p) f -> n p f", p=128)  # [784,128,128]
            NT = TBL // 128
            for b in ([] if skip_p1 else range(0, NT, 16)):
                nb = min(16, NT - b)
                ps = psump.tile([128, 16, 64], dt.float32, tag="ps")
                xt = xtp.tile([128, nb * 128], dt.bfloat16, tag="xt")
                nc.sync.dma_start(xt[:], xT_d.ap()[:, b * 128:(b + nb) * 128])
                for t in range(nb):
                    nc.tensor.matmul(ps[:, t, :],
                                     lhsT=xt[:, t * 128:(t + 1) * 128],
                                     rhs=W1t[:], start=True, stop=True)
                sb = cpp.tile([128, nb, 128], dt.bfloat16, tag="sb")
                nc.vector.tensor_copy(sb[:, :, 0:64], ps[:, :nb, :])
                nc.sync.dma_start(
                    t1v[b:b + nb].rearrange("n p f -> p n f"), sb[:])

            # ---- gather + segmented reduce over one layer ----
            def spmm(table, Sig, nfeat):
                srcA = table[BASE_A:, :]
                srcB = table[BASE_B:, :]
                qn = 0
                reg = None
                for c0 in range(0, W_total, IDXCH):
                    nw = min(IDXCH, W_total - c0)
                    it = idxp.tile([128, nw * 64], dt.int16, tag="idx")
                    nc.sync.dma_start(
                        it[0:16, :], idx_d.ap()[:, c0 * 64:(c0 + nw) * 64])
                    replicate16(it)
                    for j in range(nw):
                        w = c0 + j
                        gpos = w * WPOS
                        tile_i = gpos // TILE_POS
                        off = gpos % TILE_POS
                        if off == 0:
                            reg = regp.tile([128, TILE_POS, 128], dt.bfloat16,
                                            tag="reg")
                        src = srcA if w < nA_win else srcB
                        nc.gpsimd.dma_gather(
                            reg[:, off:off + WPOS, :], src,
                            it[:, j * 64:(j + 1) * 64], NI, NI, 128,
                            queue_num=qn % 4)
                        qn += 1
                        if off + WPOS == TILE_POS or w == W_total - 1:
                            for op in ops_by_tile.get(tile_i, []):
                                (_, o, ncols, d, g0, full) = op
                                rv = reg[:, o:o + ncols * d, 0:nfeat]
                                rv = rv.rearrange("p (n d) f -> p n f d", d=d)
                                tmp = tmpp.tile([128, MAXCOLS, nfeat],
                                                dt.float32, tag=f"tmp{nfeat}")
                                nc.vector.tensor_reduce(
                                    tmp[:, :ncols, :], rv,
                                    mybir.AxisListType.X, mybir.AluOpType.add)
                                o_ = Sig[:, g0:g0 + ncols, :]
                                nc.any.tensor_add(o_, o_, tmp[:, :ncols, :])

            # ---- P2: layer-1 spmm ----
            Sig1 = nodep.tile([128, SLOTS_P, 64], dt.float32, tag="sig1")
            nc.vector.memset(Sig1[:], 0.0)
            if not skip_spmm1:
                spmm(table1, Sig1[:], 64)

            # ---- P3: z2 = (dinv^2 * relu(Sig1)) @ W2 ----
            dv2 = dsqt[:].to_broadcast([128, SLOTS_P, 64])
            nc.vector.tensor_scalar_max(Sig1[:], Sig1[:], 0.0)
            nc.vector.tensor_tensor(Sig1[:], Sig1[:], dv2,
                                    mybir.AluOpType.mult)
            z2t = nodep.tile([128, SLOTS_P, 2], dt.float32, tag="z2")
            for j in range(2):
                w2j = w2t[:, j, :].to_broadcast(
                    [128, 64, SLOTS_P]).rearrange("p f n -> p n f")
                tmpm = nodep.tile([128, SLOTS_P, 64], dt.float32, tag="zp")
                nc.vector.tensor_tensor(tmpm[:], Sig1[:], w2j,
                                        mybir.AluOpType.mult)
                nc.vector.tensor_reduce(z2t[:, :, j], tmpm[:],
                                        mybir.AxisListType.X,
                                        mybir.AluOpType.add)
            nc.sync.dma_start(
                z2loc[:].rearrange("(s p) c -> p s c", p=128), z2t[:])

            # ---- P4: AllGather z2 (100KB) ----
            if not skip_coll:
                nc.gpsimd.collective_compute(
                    "AllGather", mybir.AluOpType.bypass,
                    replica_groups=[list(range(NCORES))],
                    ins=[z2loc.opt()], outs=[table2z.opt()])

            # ---- P5: scatter z2 into 64-wide table2 (cols 2:64 garbage) ----
            t2zv = table2z[:].rearrange("(b s p) c -> b p s c", p=128,
                                        s=SLOTS_P)
            t2v = table2[:].rearrange("(b s p) f -> b p s f", p=128,
                                      s=SLOTS_P)
            for b in range(NCORES):
                tz = cpp.tile([128, SLOTS_P, 2], dt.float32, tag="tz")
                nc.sync.dma_start(tz[:], t2zv[b])
                stage = regp.tile([128, TILE_POS, 128], dt.bfloat16,
                                  tag="reg")
                nc.vector.tensor_copy(stage[:, :SLOTS_P, 0:2], tz[:])
                nc.sync.dma_start(t2v[b], stage[:, :SLOTS_P, :])

            # ---- P6: layer-2 spmm (only cols 0:2 read) ----
            Sig2 = nodep.tile([128, SLOTS_P, 2], dt.float32, tag="sig2")
            nc.vector.memset(Sig2[:], 0.0)
            if not skip_spmm2:
                spmm(table2, Sig2[:], 2)

            # ---- P7: out = dinv * Sig2 ----
            dv1 = dint[:].to_broadcast([128, SLOTS_P, 2])
            nc.vector.tensor_tensor(Sig2[:], Sig2[:], dv1,
                                    mybir.AluOpType.mult)
            nc.sync.dma_start(
                out_d.ap().rearrange("(s p) c -> p s c", p=128), Sig2[:])

    nc.compile()
    return nc


def kernel(x, edge_index, W1, b1, W2, b2):
    x = np.asarray(x)
    edge_index = np.asarray(edge_index)
    W1 = np.asarray(W1, dtype=np.float32)
    b1 = np.asarray(b1, dtype=np.float32)
    W2 = np.asarray(W2, dtype=np.float32)
    b2 = np.asarray(b2, dtype=np.float32)
    assert np.abs(b1).max() == 0 and np.abs(b2).max() == 0

    pre = _preprocess(edge_index)
    dinv = pre["dinv"]
    nodes_all = pre["nodes_all"]

    # xT: [128, TBL] bf16, column grow(v) = dinv_v * x_v ; dummies zero
    xT = np.zeros((128, TBL), ml_dtypes.bfloat16)
    xs = (x.astype(np.float32) * dinv[:, None]).astype(ml_dtypes.bfloat16)
    xT[:, pre["grow"]] = xs.T

    W1b = W1.astype(ml_dtypes.bfloat16)
    w2rep = np.broadcast_to(W2.T[None, :, :], (128, 2, 64)).copy().astype(
        np.float32)

    nc = _build(pre)

    in_maps = []
    for p in range(NCORES):
        nf = nodes_all[p]
        dsq = np.zeros((128, SLOTS_P), np.float32)
        din = np.zeros((128, SLOTS_P), np.float32)
        valid = nf >= 0
        ranks = np.arange(SLOTS)
        parts, slots = ranks % 128, ranks // 128
        dv = np.where(valid, dinv[np.clip(nf, 0, N - 1)], 0.0)
        dsq[parts, slots] = (dv * dv).astype(np.float32)
        din[parts, slots] = dv.astype(np.float32)
        in_maps.append({
            "xT": np.asarray(xT),
            "W1b": np.asarray(W1b),
            "w2rep": w2rep,
            "dinvsq": dsq,
            "dinv1": din,
            "idxs": pre["idx_streams"][p],
        })

    res = _run(nc, in_maps)

    out = np.zeros((N, 2), np.float32)
    for p in range(NCORES):
        o = res[p]["out2"]          # row r = rank r
        nf = nodes_all[p]
        valid = nf >= 0
        out[nf[valid]] = o[np.arange(SLOTS)[valid]]
    return out.astype(x.dtype if np.issubdtype(x.dtype, np.floating)
                      else np.float32)


def _run(nc, in_maps):
    from concourse import bass_utils
    res = bass_utils.run_bass_kernel_spmd(nc, in_maps,
                                          core_ids=list(range(NCORES)))
    return res.results
